# revision 5
# baseline (speedup 1.0000x reference)
"""Trainium2 Bass kernel for a 3-layer LIF spiking MLP (CLAPP SNN eval forward).

Reference computation (T=32, IN=H=4096, L=3, OUT=10, beta=0.75, thresh=1.0):
    per step t: h = inp[t]
      for each fc layer: cur = W @ h; m = beta*m + cur; s = (m > 1); m -= s; h = s
      out layer: cur_o = out_w @ h; LIF on 10-neuron output layer
    returns (out_spks [T,10], mem_his [T,3,4096])

Key restructuring: layer ℓ's input over ALL timesteps depends only on layer
ℓ-1's spikes, so the time scan of GEMVs becomes, per layer, one GEMM over all
32 timesteps followed by a cheap elementwise LIF scan. Layers run sequentially;
time runs in parallel through the tensor engine.

Sharding: each 4096x4096 fc is row-sharded across 8 cores (512 rows each).
After each layer's LIF scan the per-core spike block ([128 part, 128] f32,
64KB) is AllGather'd so every core has the full 4096-spike input for the next
layer. The final 10-row output layer is computed redundantly on every core.

Layouts (per core c):
  - weights in DRAM pre-transposed on host: WT_l = fc_l[512c:512c+512,:].T,
    shape [4096, 512], streamed in 2MB chunks [128p, 8q, 512n] where
    k = 1024*kt + 128*q + p.
  - matmul (per k-tile K=8kt+q, m-tile m): psum[:, m, :] += lhsT.T @ rhs with
    lhsT = chunk[:, q, 128m:128m+128]  (stationary, = W^T tile, [128k, 128n])
    rhs  = spkT[:, 32K:32K+32]         (moving, [128k, 32t])
    psum tile [128, 4, 3,2] j-major: psum[p, j, t] = cur at neuron 128j+p, time t
  - LIF scan walks t with [128, 4] strided slices; spikes/membranes stored
    [128p, 4j, 32t] so the spike tile is directly the AllGather payload and
    the gathered result is directly the next layer's rhs.
"""

import numpy as np

BETA = 0.75
THRESH = 1.0
T, IN, H, L, OUT = 32, 4096, 4096, 3, 10
NCORES = 8
RS = H // NCORES          # 512 rows per core
NKT = IN // 128           # 32 k-tiles
NM = RS // 128            # 4 m-tiles per core
CHUNK_ROWS = 1024         # W^T rows per DMA chunk (2 MB)
NCHUNK = IN // CHUNK_ROWS  # 4 chunks per layer


def _build_program():
    import concourse.bacc as bacc
    import concourse.bass as bass
    import concourse.mybir as mybir
    import concourse.tile as tile

    f32 = mybir.dt.float32
    Alu = mybir.AluOpType

    nc = bacc.Bacc("TRN2", target_bir_lowering=False, debug=False,
                   num_devices=NCORES)

    # --- DRAM I/O (per-core) ---
    wts = [nc.dram_tensor(f"wt{l}", [IN, RS], f32, kind="ExternalInput")
           for l in range(L)]
    inpt = nc.dram_tensor("inpt", [128, NKT * T], f32, kind="ExternalInput")
    owt = nc.dram_tensor("owt", [128, NKT * OUT], f32, kind="ExternalInput")
    memh = nc.dram_tensor("memh", [L, 128, NM, T], f32, kind="ExternalOutput")
    ospk = nc.dram_tensor("ospk", [OUT, T], f32, kind="ExternalOutput")

    rg = [list(range(NCORES))]

    with tile.TileContext(nc) as tc:
        with (
            tc.tile_pool(name="wpool", bufs=4) as wpool,
            tc.tile_pool(name="cpool", bufs=1) as cpool,
            tc.tile_pool(name="spool", bufs=1) as spool,
            tc.tile_pool(name="pspool", bufs=1, space="PSUM") as pspool,
            tc.tile_pool(name="dpool", bufs=1, space="DRAM") as dpool,
        ):
            sb_inpt = cpool.tile([128, NKT * T], f32)
            nc.scalar.dma_start(out=sb_inpt[:], in_=inpt[:])
            sb_owt = cpool.tile([128, NKT * OUT], f32)
            nc.scalar.dma_start(out=sb_owt[:], in_=owt[:])
            zeros4 = cpool.tile([128, NM], f32)
            nc.vector.memset(zeros4[:], 0.0)
            zeros_o = cpool.tile([OUT, 1], f32)
            nc.vector.memset(zeros_o[:], 0.0)

            rhs_buf = sb_inpt
            for l in range(L):
                # ---- GEMM: cur[p, j, t] accumulated over 32 k-tiles ----
                # Each m-tile's accumulation group must own a full PSUM bank
                # (start_tensor_calc zeroes bank-granular regions), so pad the
                # last dim to 512 f32 = one 2KB bank per m-tile.
                BANK = 512
                ps = pspool.tile([128, NM, BANK], f32, name="ps", tag="ps")
                for kt in range(NCHUNK):
                    wchunk = wpool.tile([128, CHUNK_ROWS // 128, RS], f32,
                                        name="wchunk", tag="wchunk")
                    src = wts[l][kt * CHUNK_ROWS:(kt + 1) * CHUNK_ROWS, :]
                    nc.sync.dma_start(
                        out=wchunk[:],
                        in_=src.rearrange("(q p) n -> p q n", p=128),
                    )
                    for q in range(CHUNK_ROWS // 128):
                        K = kt * (CHUNK_ROWS // 128) + q
                        for m in range(NM):
                            nc.tensor.matmul(
                                ps[:, m, 0:T],
                                lhsT=wchunk[:, q, 128 * m:128 * (m + 1)],
                                rhs=rhs_buf[:, T * K:T * (K + 1)],
                                start=(K == 0),
                                stop=(K == NKT - 1),
                            )

                # ---- LIF scan over t (state in [128, 4] strided slices) ----
                spk = spool.tile([128, NM, T], f32, name=f"spk{l}")
                memb = spool.tile([128, NM, T], f32, name=f"memb{l}")
                tmp = spool.tile([128, NM], f32, name=f"tmp{l}")
                for t in range(T):
                    prev = zeros4[:] if t == 0 else memb[:, :, t - 1]
                    nc.vector.scalar_tensor_tensor(
                        tmp[:], prev, BETA, ps[:, :, t], Alu.mult, Alu.add)
                    nc.vector.tensor_scalar(
                        spk[:, :, t], tmp[:], THRESH, None, Alu.is_gt)
                    nc.vector.tensor_tensor(
                        memb[:, :, t], tmp[:], spk[:, :, t], Alu.subtract)

                nc.scalar.dma_start(out=memh[l], in_=memb[:])

                # ---- AllGather spikes so every core has the full input ----
                ag_in = dpool.tile([128, NM * T], f32, name=f"agin{l}")
                ag_out = dpool.tile([NCORES * 128, NM * T], f32,
                                    name=f"agout{l}", addr_space="Shared")
                nc.scalar.dma_start(
                    out=ag_in[:], in_=spk.rearrange("p j t -> p (j t)"))
                nc.gpsimd.collective_compute(
                    "AllGather", Alu.bypass, replica_groups=rg,
                    ins=[ag_in[:].opt()], outs=[ag_out[:].opt()])
                sb_spkT = spool.tile([128, NCORES * NM * T], f32,
                                     name=f"spkT{l}")
                nc.scalar.dma_start(
                    out=sb_spkT.rearrange("p (c f) -> p c f", c=NCORES),
                    in_=ag_out.rearrange("(c p) f -> p c f", c=NCORES),
                )
                rhs_buf = sb_spkT

            # ---- output layer: 10 neurons, computed redundantly per core ----
            ps_o = pspool.tile([OUT, T], f32, name="ps_o", tag="ps_o")
            for K in range(NKT):
                nc.tensor.matmul(
                    ps_o[:, :],
                    lhsT=sb_owt[:, OUT * K:OUT * (K + 1)],
                    rhs=rhs_buf[:, T * K:T * (K + 1)],
                    start=(K == 0),
                    stop=(K == NKT - 1),
                )
            spk_o = spool.tile([OUT, T], f32)
            mem_o = spool.tile([OUT, T], f32)
            tmp_o = spool.tile([OUT, 1], f32)
            for t in range(T):
                prev = zeros_o[:] if t == 0 else mem_o[:, t - 1:t]
                nc.vector.scalar_tensor_tensor(
                    tmp_o[:], prev, BETA, ps_o[:, t:t + 1], Alu.mult, Alu.add)
                nc.vector.tensor_scalar(
                    spk_o[:, t:t + 1], tmp_o[:], THRESH, None, Alu.is_gt)
                nc.vector.tensor_tensor(
                    mem_o[:, t:t + 1], tmp_o[:], spk_o[:, t:t + 1],
                    Alu.subtract)
            nc.scalar.dma_start(out=ospk[:], in_=spk_o[:])

    nc.compile()
    return nc


def _host_inputs(inp, fc0, fc1, fc2, out_w):
    """Per-core input maps with host-side pre-transpose/packing."""
    inp = np.asarray(inp, np.float32)
    out_w = np.asarray(out_w, np.float32)
    # inpt[p, K*32+t] = inp[t, 128K+p]
    inpt = np.ascontiguousarray(
        inp.T.reshape(NKT, 128, T).transpose(1, 0, 2).reshape(128, NKT * T))
    # owt[p, K*10+o] = out_w[o, 128K+p]
    owt = np.ascontiguousarray(
        out_w.T.reshape(NKT, 128, OUT).transpose(1, 0, 2)
        .reshape(128, NKT * OUT))
    in_maps = []
    for c in range(NCORES):
        m = {"inpt": inpt, "owt": owt}
        for l, fc in enumerate((fc0, fc1, fc2)):
            fc = np.asarray(fc, np.float32)
            m[f"wt{l}"] = np.ascontiguousarray(
                fc[c * RS:(c + 1) * RS, :].T)
        in_maps.append(m)
    return in_maps


def _assemble(results):
    """Gather per-core outputs back to full (out_spks, mem_his)."""
    mem_his = np.empty((T, L, H), np.float32)
    for c in range(NCORES):
        mh = results[c]["memh"]            # [L, 128, NM, T]
        blk = mh.transpose(3, 0, 2, 1)     # [T, L, NM, 128]
        mem_his[:, :, c * RS:(c + 1) * RS] = blk.reshape(T, L, RS)
    out_spks = np.ascontiguousarray(results[0]["ospk"].T)  # [T, OUT]
    return out_spks, mem_his


_RUN_CACHE = {}


def kernel(inp, fc0, fc1, fc2, out_w, target=None, bf=None, **_unused):
    from concourse import bass_utils

    if "nc" not in _RUN_CACHE:
        _RUN_CACHE["nc"] = _build_program()
    nc = _RUN_CACHE["nc"]
    in_maps = _host_inputs(inp, fc0, fc1, fc2, out_w)
    res = bass_utils.run_bass_kernel_spmd(nc, in_maps, list(range(NCORES)))
    return _assemble(res.results)


# revision 6
# speedup vs baseline: 1.2134x; 1.2134x over previous
"""Trainium2 Bass kernel for a 3-layer LIF spiking MLP (CLAPP SNN eval forward).

Reference computation (T=32, IN=H=4096, L=3, OUT=10, beta=0.75, thresh=1.0):
    per step t: h = inp[t]
      for each fc layer: cur = W @ h; m = beta*m + cur; s = (m > 1); m -= s; h = s
      out layer: cur_o = out_w @ h; LIF on 10-neuron output layer
    returns (out_spks [T,10], mem_his [T,3,4096])

Key restructuring: layer ℓ's input over ALL timesteps depends only on layer
ℓ-1's spikes, so the time scan of GEMVs becomes, per layer, one GEMM over all
32 timesteps followed by a cheap elementwise LIF scan. Layers run sequentially;
time runs in parallel through the tensor engine.

Precision: weights are split on host into fp16 hi + fp16 lo*2^11 parts
(w = hi + lo', lo' = fp16((w-hi)*2048)); the matmul computes
hi@s + lo'@(s*2^-11) with fp32 PSUM accumulation. The split residual is
~2^-22|w| per element (~1e-7 relative per dot), far below the minimum
spike-threshold margin of this problem instance (6.7e-6), so spike decisions
match the fp32 reference exactly. fp16 runs the PE at 1 cycle/row (4x fp32).

Matmul orientation: spikes are the STATIONARY operand ([128k, 32t] tiles,
cheap reloads), weights are the MOVING operand ([128k, 512n] fp16, 1 col/cyc)
into psum [32t, 512n], accumulated over 32 k-tiles. A DVE copy + 4 PE
transposes restore the scan-friendly [n-part, t-free] layout.

Sharding: each 4096x4096 fc is row-sharded across 8 cores (512 rows each).
After each layer's LIF scan the per-core spike block ([128, 128] fp16, 32KB)
is AllGather'd so every core has the full 4096-spike input for the next
layer. The 10-row output layer is computed redundantly on every core.
"""

import numpy as np

BETA = 0.75
THRESH = 1.0
T, IN, H, L, OUT = 32, 4096, 4096, 3, 10
NCORES = 8
RS = H // NCORES          # 512 rows per core
NKT = IN // 128           # 32 k-tiles
NM = RS // 128            # 4 m-tiles per core
CHUNK_ROWS = 1024         # W^T rows per DMA chunk (1 MB per split in fp16)
NCHUNK = IN // CHUNK_ROWS
QPC = CHUNK_ROWS // 128   # k-tiles per chunk
LO_SCALE = 2048.0         # lo split pre-scale (2^11)


def _build_program():
    import concourse.bacc as bacc
    import concourse.bass as bass
    import concourse.mybir as mybir
    import concourse.tile as tile

    f32 = mybir.dt.float32
    f16 = mybir.dt.float16
    Alu = mybir.AluOpType

    nc = bacc.Bacc("TRN2", target_bir_lowering=False, debug=False,
                   num_devices=NCORES)

    # --- DRAM I/O (per-core) ---
    wh = [nc.dram_tensor(f"wt{l}h", [IN, RS], f16, kind="ExternalInput")
          for l in range(L)]
    wl = [nc.dram_tensor(f"wt{l}l", [IN, RS], f16, kind="ExternalInput")
          for l in range(L)]
    inph = nc.dram_tensor("inph", [128, NKT * T], f16, kind="ExternalInput")
    inpl = nc.dram_tensor("inpl", [128, NKT * T], f16, kind="ExternalInput")
    inphs = nc.dram_tensor("inphs", [128, NKT * T], f16, kind="ExternalInput")
    owh = nc.dram_tensor("owh", [128, NKT * OUT], f16, kind="ExternalInput")
    owl = nc.dram_tensor("owl", [128, NKT * OUT], f16, kind="ExternalInput")
    ident_d = nc.dram_tensor("ident", [T, T], f32, kind="ExternalInput")
    memh = nc.dram_tensor("memh", [L, 128, NM, T], f32, kind="ExternalOutput")
    ospk = nc.dram_tensor("ospk", [OUT, T], f32, kind="ExternalOutput")

    rg = [list(range(NCORES))]

    with tile.TileContext(nc) as tc:
        with (
            tc.tile_pool(name="wpool", bufs=3) as wpool,
            tc.tile_pool(name="cpool", bufs=1) as cpool,
            tc.tile_pool(name="spool", bufs=1) as spool,
            tc.tile_pool(name="pspool", bufs=1, space="PSUM") as pspool,
            tc.tile_pool(name="dpool", bufs=1, space="DRAM") as dpool,
        ):
            sb_inph = cpool.tile([128, NKT * T], f16)
            nc.scalar.dma_start(out=sb_inph[:], in_=inph[:])
            sb_inpl = cpool.tile([128, NKT * T], f16)
            nc.scalar.dma_start(out=sb_inpl[:], in_=inpl[:])
            sb_inphs = cpool.tile([128, NKT * T], f16)
            nc.scalar.dma_start(out=sb_inphs[:], in_=inphs[:])
            sb_owh = cpool.tile([128, NKT * OUT], f16)
            nc.scalar.dma_start(out=sb_owh[:], in_=owh[:])
            sb_owl = cpool.tile([128, NKT * OUT], f16)
            nc.scalar.dma_start(out=sb_owl[:], in_=owl[:])
            sb_id = cpool.tile([T, T], f32)
            nc.scalar.dma_start(out=sb_id[:], in_=ident_d[:])
            zeros4 = cpool.tile([128, NM], f32)
            nc.vector.memset(zeros4[:], 0.0)
            zeros_o = cpool.tile([OUT, 1], f32)
            nc.vector.memset(zeros_o[:], 0.0)

            rhs_hi, rhs_lo = sb_inph, sb_inphs
            for l in range(L):
                # ---- GEMM into psum [32t, 512n], weights moving ----
                ps_acc = pspool.tile([T, RS], f32, name="ps_acc", tag="psacc")
                for kt in range(NCHUNK):
                    whc = wpool.tile([128, QPC, RS], f16, name="whc",
                                     tag="whc")
                    nc.sync.dma_start(
                        out=whc[:],
                        in_=wh[l][kt * CHUNK_ROWS:(kt + 1) * CHUNK_ROWS, :]
                        .rearrange("(q p) n -> p q n", p=128))
                    wlc = wpool.tile([128, QPC, RS], f16, name="wlc",
                                     tag="wlc")
                    nc.sync.dma_start(
                        out=wlc[:],
                        in_=wl[l][kt * CHUNK_ROWS:(kt + 1) * CHUNK_ROWS, :]
                        .rearrange("(q p) n -> p q n", p=128))
                    for q in range(QPC):
                        K = kt * QPC + q
                        ts_sl = slice(T * K, T * (K + 1))
                        if l == 0:
                            nc.tensor.matmul(
                                ps_acc[:], lhsT=sb_inph[:, ts_sl],
                                rhs=whc[:, q, :],
                                start=(K == 0), stop=False)
                            nc.tensor.matmul(
                                ps_acc[:], lhsT=sb_inpl[:, ts_sl],
                                rhs=whc[:, q, :],
                                start=False, stop=False)
                            nc.tensor.matmul(
                                ps_acc[:], lhsT=sb_inphs[:, ts_sl],
                                rhs=wlc[:, q, :],
                                start=False, stop=(K == NKT - 1))
                        else:
                            nc.tensor.matmul(
                                ps_acc[:], lhsT=rhs_hi[:, ts_sl],
                                rhs=whc[:, q, :],
                                start=(K == 0), stop=False)
                            nc.tensor.matmul(
                                ps_acc[:], lhsT=rhs_lo[:, ts_sl],
                                rhs=wlc[:, q, :],
                                start=False, stop=(K == NKT - 1))

                # ---- transpose to scan layout [128p, 4m, 32t] ----
                ct = spool.tile([T, RS], f32, name=f"ct{l}")
                nc.vector.tensor_copy(ct[:], ps_acc[:])
                BANK = 512
                ps2 = pspool.tile([128, NM, BANK], f32, name="ps2", tag="ps2")
                for m in range(NM):
                    nc.tensor.transpose(
                        ps2[:, m, 0:T], ct[:, 128 * m:128 * (m + 1)],
                        sb_id[:])

                # ---- LIF scan (negated membrane nm = -mem) ----
                spk = spool.tile([128, NM, T], f16, name=f"spk{l}")
                nmem = spool.tile([128, NM, T], f32, name=f"nmem{l}")
                tmp = spool.tile([128, NM], f32, name=f"tmp{l}")
                for t in range(T):
                    prev = zeros4[:] if t == 0 else nmem[:, :, t - 1]
                    nc.vector.scalar_tensor_tensor(
                        tmp[:], prev, -BETA, ps2[:, :, t], Alu.mult, Alu.add)
                    nc.vector.scalar_tensor_tensor(
                        nmem[:, :, t], tmp[:], THRESH, tmp[:],
                        Alu.is_gt, Alu.subtract)
                    nc.vector.tensor_scalar(
                        spk[:, :, t], tmp[:], THRESH, None, Alu.is_gt)

                # mem_his = -nmem
                memp = spool.tile([128, NM, T], f32, name=f"memp{l}")
                nc.vector.tensor_scalar(
                    memp[:], nmem[:], -1.0, None, Alu.mult)
                nc.scalar.dma_start(out=memh[l], in_=memp[:])

                # ---- AllGather spikes (fp16, 32KB per core) ----
                ag_in = dpool.tile([128, NM * T], f16, name=f"agin{l}")
                ag_out = dpool.tile([NCORES * 128, NM * T], f16,
                                    name=f"agout{l}", addr_space="Shared")
                nc.scalar.dma_start(
                    out=ag_in[:], in_=spk.rearrange("p j t -> p (j t)"))
                nc.gpsimd.collective_compute(
                    "AllGather", Alu.bypass, replica_groups=rg,
                    ins=[ag_in[:].opt()], outs=[ag_out[:].opt()])
                sb_spkT = spool.tile([128, NCORES * NM * T], f16,
                                     name=f"spkT{l}")
                nc.scalar.dma_start(
                    out=sb_spkT.rearrange("p (c f) -> p c f", c=NCORES),
                    in_=ag_out.rearrange("(c p) f -> p c f", c=NCORES))
                sb_spkTs = spool.tile([128, NCORES * NM * T], f16,
                                      name=f"spkTs{l}")
                nc.vector.tensor_scalar(
                    sb_spkTs[:], sb_spkT[:], 1.0 / LO_SCALE, None, Alu.mult)
                rhs_hi, rhs_lo = sb_spkT, sb_spkTs

            # ---- output layer (stationary out_w tiles, redundant/core) ----
            ps_o = pspool.tile([OUT, T], f32, name="ps_o", tag="ps_o")
            for K in range(NKT):
                os_sl = slice(OUT * K, OUT * (K + 1))
                ts_sl = slice(T * K, T * (K + 1))
                nc.tensor.matmul(
                    ps_o[:], lhsT=sb_owh[:, os_sl], rhs=rhs_hi[:, ts_sl],
                    start=(K == 0), stop=False)
                nc.tensor.matmul(
                    ps_o[:], lhsT=sb_owl[:, os_sl], rhs=rhs_lo[:, ts_sl],
                    start=False, stop=(K == NKT - 1))
            spk_o = spool.tile([OUT, T], f32)
            nmem_o = spool.tile([OUT, T], f32)
            tmp_o = spool.tile([OUT, 1], f32)
            for t in range(T):
                prev = zeros_o[:] if t == 0 else nmem_o[:, t - 1:t]
                nc.vector.scalar_tensor_tensor(
                    tmp_o[:], prev, -BETA, ps_o[:, t:t + 1],
                    Alu.mult, Alu.add)
                nc.vector.scalar_tensor_tensor(
                    nmem_o[:, t:t + 1], tmp_o[:], THRESH, tmp_o[:],
                    Alu.is_gt, Alu.subtract)
                nc.vector.tensor_scalar(
                    spk_o[:, t:t + 1], tmp_o[:], THRESH, None, Alu.is_gt)
            nc.scalar.dma_start(out=ospk[:], in_=spk_o[:])

    nc.compile()
    return nc


def _split16(a64):
    """fp16 hi/lo split: a ≈ hi + lo/2048 with lo = fp16((a-hi)*2048)."""
    hi = a64.astype(np.float16)
    lo = ((a64 - hi.astype(np.float64)) * LO_SCALE).astype(np.float16)
    return hi, lo


def _pack_kt(mat_T64):
    """[4096, cols] -> [128, 32*cols] packed so col K*cols+c = mat[128K+p, c]."""
    cols = mat_T64.shape[1]
    return np.ascontiguousarray(
        mat_T64.reshape(NKT, 128, cols).transpose(1, 0, 2)
        .reshape(128, NKT * cols))


def _host_inputs(inp, fc0, fc1, fc2, out_w):
    """Per-core input maps with host-side packing and hi/lo splitting."""
    inp64 = np.asarray(inp, np.float64)
    ow64 = np.asarray(out_w, np.float64)

    xT = inp64.T                          # [4096, 32]
    xhi = xT.astype(np.float16)
    xlo = (xT - xhi.astype(np.float64)).astype(np.float16)
    xhis = (xhi.astype(np.float64) / LO_SCALE).astype(np.float16)
    inph = _pack_kt(xhi.astype(np.float64)).astype(np.float16)
    inpl = _pack_kt(xlo.astype(np.float64)).astype(np.float16)
    inphs = _pack_kt(xhis.astype(np.float64)).astype(np.float16)

    owhi, owlo = _split16(ow64.T)         # [4096, 10] each
    owh = _pack_kt(owhi.astype(np.float64)).astype(np.float16)
    owl = _pack_kt(owlo.astype(np.float64)).astype(np.float16)

    ident = np.eye(T, dtype=np.float32)

    shared = {"inph": inph, "inpl": inpl, "inphs": inphs,
              "owh": owh, "owl": owl, "ident": ident}
    in_maps = []
    for c in range(NCORES):
        m = dict(shared)
        for l, fc in enumerate((fc0, fc1, fc2)):
            wt = np.asarray(fc, np.float64)[c * RS:(c + 1) * RS, :].T
            hi, lo = _split16(np.ascontiguousarray(wt))
            m[f"wt{l}h"] = hi
            m[f"wt{l}l"] = lo
        in_maps.append(m)
    return in_maps


def _assemble(results):
    """Gather per-core outputs back to full (out_spks, mem_his)."""
    mem_his = np.empty((T, L, H), np.float32)
    for c in range(NCORES):
        mh = results[c]["memh"]            # [L, 128, NM, T]
        blk = mh.transpose(3, 0, 2, 1)     # [T, L, NM, 128]
        mem_his[:, :, c * RS:(c + 1) * RS] = blk.reshape(T, L, RS)
    out_spks = np.ascontiguousarray(results[0]["ospk"].T)  # [T, OUT]
    return out_spks, mem_his


_RUN_CACHE = {}


def kernel(inp, fc0, fc1, fc2, out_w, target=None, bf=None, **_unused):
    from concourse import bass_utils

    if "nc" not in _RUN_CACHE:
        _RUN_CACHE["nc"] = _build_program()
    nc = _RUN_CACHE["nc"]
    in_maps = _host_inputs(inp, fc0, fc1, fc2, out_w)
    res = bass_utils.run_bass_kernel_spmd(nc, in_maps, list(range(NCORES)))
    return _assemble(res.results)


# revision 10
# speedup vs baseline: 1.4693x; 1.2108x over previous
"""Trainium2 Bass kernel for a 3-layer LIF spiking MLP (CLAPP SNN eval forward).

Reference computation (T=32, IN=H=4096, L=3, OUT=10, beta=0.75, thresh=1.0):
    per step t: h = inp[t]
      for each fc layer: cur = W @ h; m = beta*m + cur; s = (m > 1); m -= s; h = s
      out layer: cur_o = out_w @ h; LIF on 10-neuron output layer
    returns (out_spks [T,10], mem_his [T,3,4096])

Key restructuring: layer ℓ's input over ALL timesteps depends only on layer
ℓ-1's spikes, so the time scan of GEMVs becomes, per layer, one GEMM over all
32 timesteps followed by a cheap elementwise LIF scan. Layers run sequentially;
time runs in parallel through the tensor engine.

Precision: weights are split on host into fp16 hi + fp16 lo*2^11 parts
(w = hi + lo', lo' = fp16((w-hi)*2048)); the matmul computes
hi@s + lo'@(s*2^-11) with fp32 PSUM accumulation. The split residual is
~2^-22|w| per element (~1e-7 relative per dot), far below the minimum
spike-threshold margin of this problem instance (6.7e-6), so spike decisions
match the fp32 reference exactly. fp16 runs the PE at 1 cycle/row (4x fp32).

Matmul orientation: spikes are the STATIONARY operand ([128k, 32t] tiles,
cheap reloads), weights are the MOVING operand ([128k, 512n] fp16, 1 col/cyc)
into psum [32t, 512n], accumulated over 32 k-tiles. A DVE copy + 4 PE
transposes restore the scan-friendly [n-part, t-free] layout.

Sharding: each 4096x4096 fc is row-sharded across 8 cores (512 rows each).
After each layer's LIF scan the per-core spike block ([128, 128] fp16, 32KB)
is AllGather'd so every core has the full 4096-spike input for the next
layer. The 10-row output layer is computed redundantly on every core.
"""

import numpy as np

BETA = 0.75
THRESH = 1.0
T, IN, H, L, OUT = 32, 4096, 4096, 3, 10
NCORES = 8
RS = H // NCORES          # 512 rows per core
NKT = IN // 128           # 32 k-tiles
NM = RS // 128            # 4 m-tiles per core
CHUNK_ROWS = 1024         # W^T rows per DMA chunk (1 MB per split in fp16)
NCHUNK = IN // CHUNK_ROWS
QPC = CHUNK_ROWS // 128   # k-tiles per chunk
LO_SCALE = 2048.0         # lo split pre-scale (2^11)


def _build_program():
    import concourse.bacc as bacc
    import concourse.bass as bass
    import concourse.mybir as mybir
    import concourse.tile as tile

    f32 = mybir.dt.float32
    f16 = mybir.dt.float16
    Alu = mybir.AluOpType

    nc = bacc.Bacc("TRN2", target_bir_lowering=False, debug=False,
                   num_devices=NCORES)

    # --- DRAM I/O (per-core) ---
    wh = [nc.dram_tensor(f"wt{l}h", [IN, RS], f16, kind="ExternalInput")
          for l in range(L)]
    wl = [nc.dram_tensor(f"wt{l}l", [IN, RS], f16, kind="ExternalInput")
          for l in range(L)]
    inph = nc.dram_tensor("inph", [128, NKT * T], f16, kind="ExternalInput")
    inpl = nc.dram_tensor("inpl", [128, NKT * T], f16, kind="ExternalInput")
    inphs = nc.dram_tensor("inphs", [128, NKT * T], f16, kind="ExternalInput")
    owh = nc.dram_tensor("owh", [128, NKT * OUT], f16, kind="ExternalInput")
    owl = nc.dram_tensor("owl", [128, NKT * OUT], f16, kind="ExternalInput")
    ident_d = nc.dram_tensor("ident", [T, T], f32, kind="ExternalInput")
    memh = nc.dram_tensor("memh", [L, 128, NM, T], f32, kind="ExternalOutput")
    ospk = nc.dram_tensor("ospk", [OUT, T], f32, kind="ExternalOutput")

    rg = [list(range(NCORES))]

    with tile.TileContext(nc) as tc:
        with (
            tc.tile_pool(name="wpool", bufs=5) as wpool,
            tc.tile_pool(name="cpool", bufs=1) as cpool,
            tc.tile_pool(name="spool", bufs=1) as spool,
            tc.tile_pool(name="pspool", bufs=1, space="PSUM") as pspool,
            tc.tile_pool(name="dpool", bufs=1, space="DRAM") as dpool,
        ):
            # Warmup collective: the first ncfw collective of a NEFF pays a
            # large cold-start (~50us observed); burn it at t=0 under the
            # layer-0 weight DMA instead of on the critical path.
            warm = cpool.tile([128, 8], f16)
            nc.gpsimd.memset(warm[:], 0.0)
            agw_in = dpool.tile([128, 8], f16, name="agwin")
            agw_out = dpool.tile([NCORES * 128, 8], f16, name="agwout",
                                 addr_space="Shared")
            nc.gpsimd.dma_start(out=agw_in[:], in_=warm[:])
            nc.gpsimd.collective_compute(
                "AllGather", Alu.bypass, replica_groups=rg,
                ins=[agw_in[:].opt()], outs=[agw_out[:].opt()])

            sb_inph = cpool.tile([128, NKT * T], f16)
            nc.gpsimd.dma_start(out=sb_inph[:], in_=inph[:])
            sb_inpl = cpool.tile([128, NKT * T], f16)
            nc.gpsimd.dma_start(out=sb_inpl[:], in_=inpl[:])
            sb_inphs = cpool.tile([128, NKT * T], f16)
            nc.gpsimd.dma_start(out=sb_inphs[:], in_=inphs[:])
            sb_owh = cpool.tile([128, NKT * OUT], f16)
            nc.gpsimd.dma_start(out=sb_owh[:], in_=owh[:])
            sb_owl = cpool.tile([128, NKT * OUT], f16)
            nc.gpsimd.dma_start(out=sb_owl[:], in_=owl[:])
            sb_id = cpool.tile([T, T], f32)
            nc.gpsimd.dma_start(out=sb_id[:], in_=ident_d[:])
            zeros4 = cpool.tile([128, NM], f32)
            nc.vector.memset(zeros4[:], 0.0)
            zeros_o = cpool.tile([OUT, 1], f32)
            nc.vector.memset(zeros_o[:], 0.0)

            rhs_hi, rhs_lo = sb_inph, sb_inphs
            for l in range(L):
                # ---- GEMM into psum [32t, 512n], weights moving ----
                ps_acc = pspool.tile([T, RS], f32, name="ps_acc", tag="psacc")
                for kt in range(NCHUNK):
                    whc = wpool.tile([128, QPC, RS], f16, name="whc",
                                     tag="whc")
                    nc.sync.dma_start(
                        out=whc[:],
                        in_=wh[l][kt * CHUNK_ROWS:(kt + 1) * CHUNK_ROWS, :]
                        .rearrange("(q p) n -> p q n", p=128))
                    wlc = wpool.tile([128, QPC, RS], f16, name="wlc",
                                     tag="wlc")
                    # lo chunks ride the second HWDGE ring (ACT) so the two
                    # 1MB streams overlap; one ring serializes at ~290 GB/s.
                    nc.scalar.dma_start(
                        out=wlc[:],
                        in_=wl[l][kt * CHUNK_ROWS:(kt + 1) * CHUNK_ROWS, :]
                        .rearrange("(q p) n -> p q n", p=128))
                    for q in range(QPC):
                        K = kt * QPC + q
                        ts_sl = slice(T * K, T * (K + 1))
                        if l == 0:
                            nc.tensor.matmul(
                                ps_acc[:], lhsT=sb_inph[:, ts_sl],
                                rhs=whc[:, q, :],
                                start=(K == 0), stop=False)
                            nc.tensor.matmul(
                                ps_acc[:], lhsT=sb_inpl[:, ts_sl],
                                rhs=whc[:, q, :],
                                start=False, stop=False)
                            nc.tensor.matmul(
                                ps_acc[:], lhsT=sb_inphs[:, ts_sl],
                                rhs=wlc[:, q, :],
                                start=False, stop=(K == NKT - 1))
                        else:
                            nc.tensor.matmul(
                                ps_acc[:], lhsT=rhs_hi[:, ts_sl],
                                rhs=whc[:, q, :],
                                start=(K == 0), stop=False)
                            nc.tensor.matmul(
                                ps_acc[:], lhsT=rhs_lo[:, ts_sl],
                                rhs=wlc[:, q, :],
                                start=False, stop=(K == NKT - 1))

                # ---- transpose to scan layout [128p, 4m, 32t] ----
                ct = spool.tile([T, RS], f32, name=f"ct{l}")
                nc.vector.tensor_copy(ct[:], ps_acc[:])
                BANK = 512
                ps2 = pspool.tile([128, NM, BANK], f32, name="ps2", tag="ps2")
                for m in range(NM):
                    nc.tensor.transpose(
                        ps2[:, m, 0:T], ct[:, 128 * m:128 * (m + 1)],
                        sb_id[:])

                # ---- LIF scan (negated membrane nm = -mem) ----
                # DVE runs the 2-op recurrence chain; GpSimd extracts spikes
                # off-chain from tmp (double-buffered so the DVE's next step
                # doesn't WAR-stall on the GpSimd read).
                spk = spool.tile([128, NM, T], f16, name=f"spk{l}")
                nmem = spool.tile([128, NM, T], f32, name=f"nmem{l}")
                tmp = spool.tile([128, NM, 2], f32, name=f"tmp{l}")
                for t in range(T):
                    prev = zeros4[:] if t == 0 else nmem[:, :, t - 1]
                    tb = tmp[:, :, t % 2]
                    nc.vector.scalar_tensor_tensor(
                        tb, prev, -BETA, ps2[:, :, t], Alu.mult, Alu.add)
                    nc.vector.scalar_tensor_tensor(
                        nmem[:, :, t], tb, THRESH, tb,
                        Alu.is_gt, Alu.subtract)
                    nc.gpsimd.tensor_scalar(
                        spk[:, :, t], tb, THRESH, None, Alu.is_gt)

                # ---- AllGather spikes (fp16, 32KB per core) ----
                ag_in = dpool.tile([128, NM * T], f16, name=f"agin{l}")
                ag_out = dpool.tile([NCORES * 128, NM * T], f16,
                                    name=f"agout{l}", addr_space="Shared")
                nc.gpsimd.dma_start(
                    out=ag_in[:], in_=spk.rearrange("p j t -> p (j t)"))
                nc.gpsimd.collective_compute(
                    "AllGather", Alu.bypass, replica_groups=rg,
                    ins=[ag_in[:].opt()], outs=[ag_out[:].opt()])

                # mem_his = -nmem (off the critical path, after AG trigger)
                memp = spool.tile([128, NM, T], f32, name=f"memp{l}")
                nc.vector.tensor_scalar(
                    memp[:], nmem[:], -1.0, None, Alu.mult)
                nc.gpsimd.dma_start(out=memh[l], in_=memp[:])

                sb_spkT = spool.tile([128, NCORES * NM * T], f16,
                                     name=f"spkT{l}")
                nc.gpsimd.dma_start(
                    out=sb_spkT.rearrange("p (c f) -> p c f", c=NCORES),
                    in_=ag_out.rearrange("(c p) f -> p c f", c=NCORES))
                sb_spkTs = spool.tile([128, NCORES * NM * T], f16,
                                      name=f"spkTs{l}")
                nc.vector.tensor_scalar(
                    sb_spkTs[:], sb_spkT[:], 1.0 / LO_SCALE, None, Alu.mult)
                rhs_hi, rhs_lo = sb_spkT, sb_spkTs

            # ---- output layer (stationary out_w tiles, redundant/core) ----
            ps_o = pspool.tile([OUT, T], f32, name="ps_o", tag="ps_o")
            for K in range(NKT):
                os_sl = slice(OUT * K, OUT * (K + 1))
                ts_sl = slice(T * K, T * (K + 1))
                nc.tensor.matmul(
                    ps_o[:], lhsT=sb_owh[:, os_sl], rhs=rhs_hi[:, ts_sl],
                    start=(K == 0), stop=False)
                nc.tensor.matmul(
                    ps_o[:], lhsT=sb_owl[:, os_sl], rhs=rhs_lo[:, ts_sl],
                    start=False, stop=(K == NKT - 1))
            spk_o = spool.tile([OUT, T], f32)
            nmem_o = spool.tile([OUT, T], f32)
            tmp_o = spool.tile([OUT, 2], f32)
            for t in range(T):
                prev = zeros_o[:] if t == 0 else nmem_o[:, t - 1:t]
                tb = tmp_o[:, t % 2:t % 2 + 1]
                nc.vector.scalar_tensor_tensor(
                    tb, prev, -BETA, ps_o[:, t:t + 1],
                    Alu.mult, Alu.add)
                nc.vector.scalar_tensor_tensor(
                    nmem_o[:, t:t + 1], tb, THRESH, tb,
                    Alu.is_gt, Alu.subtract)
                nc.gpsimd.tensor_scalar(
                    spk_o[:, t:t + 1], tb, THRESH, None, Alu.is_gt)
            nc.gpsimd.dma_start(out=ospk[:], in_=spk_o[:])

    nc.compile()
    return nc


def _split16(a64):
    """fp16 hi/lo split: a ≈ hi + lo/2048 with lo = fp16((a-hi)*2048)."""
    hi = a64.astype(np.float16)
    lo = ((a64 - hi.astype(np.float64)) * LO_SCALE).astype(np.float16)
    return hi, lo


def _pack_kt(mat_T64):
    """[4096, cols] -> [128, 32*cols] packed so col K*cols+c = mat[128K+p, c]."""
    cols = mat_T64.shape[1]
    return np.ascontiguousarray(
        mat_T64.reshape(NKT, 128, cols).transpose(1, 0, 2)
        .reshape(128, NKT * cols))


def _host_inputs(inp, fc0, fc1, fc2, out_w):
    """Per-core input maps with host-side packing and hi/lo splitting."""
    inp64 = np.asarray(inp, np.float64)
    ow64 = np.asarray(out_w, np.float64)

    xT = inp64.T                          # [4096, 32]
    xhi = xT.astype(np.float16)
    xlo = (xT - xhi.astype(np.float64)).astype(np.float16)
    xhis = (xhi.astype(np.float64) / LO_SCALE).astype(np.float16)
    inph = _pack_kt(xhi.astype(np.float64)).astype(np.float16)
    inpl = _pack_kt(xlo.astype(np.float64)).astype(np.float16)
    inphs = _pack_kt(xhis.astype(np.float64)).astype(np.float16)

    owhi, owlo = _split16(ow64.T)         # [4096, 10] each
    owh = _pack_kt(owhi.astype(np.float64)).astype(np.float16)
    owl = _pack_kt(owlo.astype(np.float64)).astype(np.float16)

    ident = np.eye(T, dtype=np.float32)

    shared = {"inph": inph, "inpl": inpl, "inphs": inphs,
              "owh": owh, "owl": owl, "ident": ident}
    in_maps = []
    for c in range(NCORES):
        m = dict(shared)
        for l, fc in enumerate((fc0, fc1, fc2)):
            wt = np.asarray(fc, np.float64)[c * RS:(c + 1) * RS, :].T
            hi, lo = _split16(np.ascontiguousarray(wt))
            m[f"wt{l}h"] = hi
            m[f"wt{l}l"] = lo
        in_maps.append(m)
    return in_maps


def _assemble(results):
    """Gather per-core outputs back to full (out_spks, mem_his)."""
    mem_his = np.empty((T, L, H), np.float32)
    for c in range(NCORES):
        mh = results[c]["memh"]            # [L, 128, NM, T]
        blk = mh.transpose(3, 0, 2, 1)     # [T, L, NM, 128]
        mem_his[:, :, c * RS:(c + 1) * RS] = blk.reshape(T, L, RS)
    out_spks = np.ascontiguousarray(results[0]["ospk"].T)  # [T, OUT]
    return out_spks, mem_his


_RUN_CACHE = {}


def kernel(inp, fc0, fc1, fc2, out_w, target=None, bf=None, **_unused):
    from concourse import bass_utils

    if "nc" not in _RUN_CACHE:
        _RUN_CACHE["nc"] = _build_program()
    nc = _RUN_CACHE["nc"]
    in_maps = _host_inputs(inp, fc0, fc1, fc2, out_w)
    res = bass_utils.run_bass_kernel_spmd(nc, in_maps, list(range(NCORES)))
    return _assemble(res.results)


# revision 17
# speedup vs baseline: 1.5527x; 1.0567x over previous
"""Trainium2 Bass kernel for a 3-layer LIF spiking MLP (CLAPP SNN eval forward).

Reference computation (T=32, IN=H=4096, L=3, OUT=10, beta=0.75, thresh=1.0):
    per step t: h = inp[t]
      for each fc layer: cur = W @ h; m = beta*m + cur; s = (m > 1); m -= s; h = s
      out layer: cur_o = out_w @ h; LIF on 10-neuron output layer
    returns (out_spks [T,10], mem_his [T,3,4096])

Key restructuring: layer ℓ's input over ALL timesteps depends only on layer
ℓ-1's spikes, so the time scan of GEMVs becomes, per layer, one GEMM over all
32 timesteps followed by a cheap elementwise LIF scan. Layers run sequentially;
time runs in parallel through the tensor engine.

Precision: weights are split on host into fp16 hi + fp16 lo*2^11 parts
(w = hi + lo', lo' = fp16((w-hi)*2048)); the matmul computes
hi@s + lo'@(s*2^-11) with fp32 PSUM accumulation. The split residual is
~2^-22|w| per element (~1e-7 relative per dot), far below the minimum
spike-threshold margin of this problem instance (6.7e-6), so spike decisions
match the fp32 reference exactly. fp16 runs the PE at 1 cycle/row (4x fp32).

Matmul orientation: spikes are the STATIONARY operand ([128k, 32t] tiles,
cheap reloads), weights are the MOVING operand ([128k, 512n] fp16, 1 col/cyc)
into psum [32t, 512n], accumulated over 32 k-tiles. A DVE copy + 4 PE
transposes restore the scan-friendly [n-part, t-free] layout.

Sharding: each 4096x4096 fc is row-sharded across 8 cores (512 rows each).
After each layer's LIF scan the per-core spike block ([128, 128] fp16, 32KB)
is AllGather'd so every core has the full 4096-spike input for the next
layer. The 10-row output layer is computed redundantly on every core.
"""

import numpy as np

BETA = 0.75
THRESH = 1.0
T, IN, H, L, OUT = 32, 4096, 4096, 3, 10
NCORES = 8
RS = H // NCORES          # 512 rows per core
NKT = IN // 128           # 32 k-tiles
NM = RS // 128            # 4 m-tiles per core
CHUNK_ROWS = 2048         # W^T rows per DMA chunk (2 MB per split in fp16)
NCHUNK = IN // CHUNK_ROWS
QPC = CHUNK_ROWS // 128   # k-tiles per chunk
LO_SCALE = 2048.0         # lo split pre-scale (2^11)


def _build_program():
    import concourse.bacc as bacc
    import concourse.bass as bass
    import concourse.mybir as mybir
    import concourse.tile as tile

    f32 = mybir.dt.float32
    f16 = mybir.dt.float16
    Alu = mybir.AluOpType

    nc = bacc.Bacc("TRN2", target_bir_lowering=False, debug=False,
                   num_devices=NCORES)

    # --- DRAM I/O (per-core) ---
    # Weights are pre-swizzled on host to [NCHUNK, 128, QPC*RS] so each
    # partition's chunk data is one contiguous 16KB run (fast descriptors).
    wh = [nc.dram_tensor(f"wt{l}h", [NCHUNK, 128, QPC * RS], f16,
                         kind="ExternalInput") for l in range(L)]
    wl = [nc.dram_tensor(f"wt{l}l", [NCHUNK, 128, QPC * RS], f16,
                         kind="ExternalInput") for l in range(L)]
    inph = nc.dram_tensor("inph", [128, NKT * T], f16, kind="ExternalInput")
    inpl = nc.dram_tensor("inpl", [128, NKT * T], f16, kind="ExternalInput")
    inphs = nc.dram_tensor("inphs", [128, NKT * T], f16, kind="ExternalInput")
    owh = nc.dram_tensor("owh", [128, NKT * OUT], f16, kind="ExternalInput")
    owl = nc.dram_tensor("owl", [128, NKT * OUT], f16, kind="ExternalInput")
    ident_d = nc.dram_tensor("ident", [T, T], f32, kind="ExternalInput")
    memh = nc.dram_tensor("memh", [L, 128, NM, T], f32, kind="ExternalOutput")
    ospk = nc.dram_tensor("ospk", [OUT, T], f32, kind="ExternalOutput")

    rg = [list(range(NCORES))]

    with tile.TileContext(nc) as tc:
        with (
            tc.tile_pool(name="wpool", bufs=3) as wpool,
            tc.tile_pool(name="cpool", bufs=1) as cpool,
            tc.tile_pool(name="spool", bufs=1) as spool,
            tc.tile_pool(name="pspool", bufs=1, space="PSUM") as pspool,
            tc.tile_pool(name="dpool", bufs=1, space="DRAM") as dpool,
        ):
            # Warmup collective: the first ncfw collective of a NEFF pays a
            # large cold-start (~50us observed); burn it at t=0 under the
            # layer-0 weight DMA instead of on the critical path. Gathers an
            # uninitialized DRAM tile — no deps, fires immediately.
            agw_in = dpool.tile([128, 8], f16, name="agwin")
            agw_out = dpool.tile([NCORES * 128, 8], f16, name="agwout",
                                 addr_space="Shared")
            nc.gpsimd.collective_compute(
                "AllGather", Alu.bypass, replica_groups=rg,
                ins=[agw_in[:].opt()], outs=[agw_out[:].opt()])

            sb_inph = cpool.tile([128, NKT * T], f16)
            nc.gpsimd.dma_start(out=sb_inph[:], in_=inph[:])
            sb_inpl = cpool.tile([128, NKT * T], f16)
            nc.gpsimd.dma_start(out=sb_inpl[:], in_=inpl[:])
            sb_inphs = cpool.tile([128, NKT * T], f16)
            nc.gpsimd.dma_start(out=sb_inphs[:], in_=inphs[:])
            sb_owh = cpool.tile([128, NKT * OUT], f16)
            nc.gpsimd.dma_start(out=sb_owh[:], in_=owh[:])
            sb_owl = cpool.tile([128, NKT * OUT], f16)
            nc.gpsimd.dma_start(out=sb_owl[:], in_=owl[:])
            sb_id = cpool.tile([T, T], f32)
            nc.gpsimd.dma_start(out=sb_id[:], in_=ident_d[:])
            zeros4 = cpool.tile([128, NM], f32)
            nc.vector.memset(zeros4[:], 0.0)
            zeros_o = cpool.tile([OUT, 1], f32)
            nc.vector.memset(zeros_o[:], 0.0)

            rhs_hi, rhs_lo = sb_inph, sb_inphs
            for l in range(L):
                # ---- GEMM into psum [32t, 512n], weights moving ----
                ps_acc = pspool.tile([T, RS], f32, name="ps_acc", tag="psacc")
                for kt in range(NCHUNK):
                    whc = wpool.tile([128, QPC, RS], f16, name="whc",
                                     tag="whc")
                    nc.sync.dma_start(
                        out=whc[:],
                        in_=wh[l][kt].rearrange("p (q n) -> p q n", q=QPC))
                    wlc = wpool.tile([128, QPC, RS], f16, name="wlc",
                                     tag="wlc")
                    # lo chunks ride the second HWDGE ring (ACT) so the two
                    # 2MB streams overlap; one ring serializes at ~290 GB/s.
                    nc.scalar.dma_start(
                        out=wlc[:],
                        in_=wl[l][kt].rearrange("p (q n) -> p q n", q=QPC))
                    for q in range(QPC):
                        K = kt * QPC + q
                        ts_sl = slice(T * K, T * (K + 1))
                        if l == 0:
                            nc.tensor.matmul(
                                ps_acc[:], lhsT=sb_inph[:, ts_sl],
                                rhs=whc[:, q, :],
                                start=(K == 0), stop=False)
                            nc.tensor.matmul(
                                ps_acc[:], lhsT=sb_inpl[:, ts_sl],
                                rhs=whc[:, q, :],
                                start=False, stop=False)
                            nc.tensor.matmul(
                                ps_acc[:], lhsT=sb_inphs[:, ts_sl],
                                rhs=wlc[:, q, :],
                                start=False, stop=(K == NKT - 1))
                        else:
                            nc.tensor.matmul(
                                ps_acc[:], lhsT=rhs_hi[:, ts_sl],
                                rhs=whc[:, q, :],
                                start=(K == 0), stop=False)
                            nc.tensor.matmul(
                                ps_acc[:], lhsT=rhs_lo[:, ts_sl],
                                rhs=wlc[:, q, :],
                                start=False, stop=(K == NKT - 1))

                # ---- transpose to scan layout [128p, 4m, 32t] ----
                ct = spool.tile([T, RS], f32, name=f"ct{l}")
                nc.vector.tensor_copy(ct[:], ps_acc[:])
                BANK = 512
                ps2 = pspool.tile([128, NM, BANK], f32, name="ps2", tag="ps2")
                for m in range(NM):
                    nc.tensor.transpose(
                        ps2[:, m, 0:T], ct[:, 128 * m:128 * (m + 1)],
                        sb_id[:])

                # ---- LIF scan (negated membrane nm = -mem) ----
                # DVE runs the 2-op recurrence chain; GpSimd extracts spikes
                # off-chain from tmp (double-buffered so the DVE's next step
                # doesn't WAR-stall on the GpSimd read).
                spk = spool.tile([128, NM, T], f16, name=f"spk{l}")
                nmem = spool.tile([128, NM, T], f32, name=f"nmem{l}")
                tmp = spool.tile([128, NM, 4], f32, name=f"tmp{l}")
                cur_s = spool.tile([128, NM, T], f32, name=f"cur_s{l}")
                nc.vector.tensor_copy(cur_s[:], ps2[:, :, 0:T])
                for t in range(T):
                    prev = zeros4[:] if t == 0 else nmem[:, :, t - 1]
                    tb = tmp[:, :, t % 4]
                    nc.vector.scalar_tensor_tensor(
                        tb, prev, -BETA, cur_s[:, :, t], Alu.mult, Alu.add)
                    nc.vector.scalar_tensor_tensor(
                        nmem[:, :, t], tb, THRESH, tb,
                        Alu.is_gt, Alu.subtract)
                    nc.gpsimd.tensor_scalar(
                        spk[:, :, t], tb, THRESH, None, Alu.is_gt)

                # ---- AllGather spikes (fp16, 32KB per core) ----
                ag_in = dpool.tile([128, NM * T], f16, name=f"agin{l}")
                ag_out = dpool.tile([NCORES * 128, NM * T], f16,
                                    name=f"agout{l}", addr_space="Shared")
                nc.gpsimd.dma_start(
                    out=ag_in[:], in_=spk.rearrange("p j t -> p (j t)"))
                nc.gpsimd.collective_compute(
                    "AllGather", Alu.bypass, replica_groups=rg,
                    ins=[ag_in[:].opt()], outs=[ag_out[:].opt()])

                # mem_his = -nmem (off the critical path, after AG trigger)
                memp = spool.tile([128, NM, T], f32, name=f"memp{l}")
                nc.vector.tensor_scalar(
                    memp[:], nmem[:], -1.0, None, Alu.mult)
                nc.gpsimd.dma_start(out=memh[l], in_=memp[:])

                sb_spkT = spool.tile([128, NCORES * NM * T], f16,
                                     name=f"spkT{l}")
                nc.gpsimd.dma_start(
                    out=sb_spkT.rearrange("p (c f) -> p c f", c=NCORES),
                    in_=ag_out.rearrange("(c p) f -> p c f", c=NCORES))
                sb_spkTs = spool.tile([128, NCORES * NM * T], f16,
                                      name=f"spkTs{l}")
                nc.vector.tensor_scalar(
                    sb_spkTs[:], sb_spkT[:], 1.0 / LO_SCALE, None, Alu.mult)
                rhs_hi, rhs_lo = sb_spkT, sb_spkTs

            # ---- output layer (stationary out_w tiles, redundant/core) ----
            ps_o = pspool.tile([OUT, T], f32, name="ps_o", tag="ps_o")
            for K in range(NKT):
                os_sl = slice(OUT * K, OUT * (K + 1))
                ts_sl = slice(T * K, T * (K + 1))
                nc.tensor.matmul(
                    ps_o[:], lhsT=sb_owh[:, os_sl], rhs=rhs_hi[:, ts_sl],
                    start=(K == 0), stop=False)
                nc.tensor.matmul(
                    ps_o[:], lhsT=sb_owl[:, os_sl], rhs=rhs_lo[:, ts_sl],
                    start=False, stop=(K == NKT - 1))
            spk_o = spool.tile([OUT, T], f32)
            nmem_o = spool.tile([OUT, T], f32)
            tmp_o = spool.tile([OUT, 2], f32)
            for t in range(T):
                prev = zeros_o[:] if t == 0 else nmem_o[:, t - 1:t]
                tb = tmp_o[:, t % 2:t % 2 + 1]
                nc.vector.scalar_tensor_tensor(
                    tb, prev, -BETA, ps_o[:, t:t + 1],
                    Alu.mult, Alu.add)
                nc.vector.scalar_tensor_tensor(
                    nmem_o[:, t:t + 1], tb, THRESH, tb,
                    Alu.is_gt, Alu.subtract)
                nc.gpsimd.tensor_scalar(
                    spk_o[:, t:t + 1], tb, THRESH, None, Alu.is_gt)
            nc.gpsimd.dma_start(out=ospk[:], in_=spk_o[:])

    nc.compile()
    return nc


def _split16(a64):
    """fp16 hi/lo split: a ≈ hi + lo/2048 with lo = fp16((a-hi)*2048)."""
    hi = a64.astype(np.float16)
    lo = ((a64 - hi.astype(np.float64)) * LO_SCALE).astype(np.float16)
    return hi, lo


def _pack_kt(mat_T64):
    """[4096, cols] -> [128, 32*cols] packed so col K*cols+c = mat[128K+p, c]."""
    cols = mat_T64.shape[1]
    return np.ascontiguousarray(
        mat_T64.reshape(NKT, 128, cols).transpose(1, 0, 2)
        .reshape(128, NKT * cols))


def _host_inputs(inp, fc0, fc1, fc2, out_w):
    """Per-core input maps with host-side packing and hi/lo splitting."""
    inp64 = np.asarray(inp, np.float64)
    ow64 = np.asarray(out_w, np.float64)

    xT = inp64.T                          # [4096, 32]
    xhi = xT.astype(np.float16)
    xlo = (xT - xhi.astype(np.float64)).astype(np.float16)
    xhis = (xhi.astype(np.float64) / LO_SCALE).astype(np.float16)
    inph = _pack_kt(xhi.astype(np.float64)).astype(np.float16)
    inpl = _pack_kt(xlo.astype(np.float64)).astype(np.float16)
    inphs = _pack_kt(xhis.astype(np.float64)).astype(np.float16)

    owhi, owlo = _split16(ow64.T)         # [4096, 10] each
    owh = _pack_kt(owhi.astype(np.float64)).astype(np.float16)
    owl = _pack_kt(owlo.astype(np.float64)).astype(np.float16)

    ident = np.eye(T, dtype=np.float32)

    shared = {"inph": inph, "inpl": inpl, "inphs": inphs,
              "owh": owh, "owl": owl, "ident": ident}
    def swizzle(w16):
        # [IN, RS] -> [NCHUNK, 128, QPC*RS]: chunk kt row 128q+p -> [kt, p, q]
        return np.ascontiguousarray(
            w16.reshape(NCHUNK, QPC, 128, RS).transpose(0, 2, 1, 3)
            .reshape(NCHUNK, 128, QPC * RS))

    in_maps = []
    for c in range(NCORES):
        m = dict(shared)
        for l, fc in enumerate((fc0, fc1, fc2)):
            wt = np.asarray(fc, np.float64)[c * RS:(c + 1) * RS, :].T
            hi, lo = _split16(np.ascontiguousarray(wt))
            m[f"wt{l}h"] = swizzle(hi)
            m[f"wt{l}l"] = swizzle(lo)
        in_maps.append(m)
    return in_maps


def _assemble(results):
    """Gather per-core outputs back to full (out_spks, mem_his)."""
    mem_his = np.empty((T, L, H), np.float32)
    for c in range(NCORES):
        mh = results[c]["memh"]            # [L, 128, NM, T]
        blk = mh.transpose(3, 0, 2, 1)     # [T, L, NM, 128]
        mem_his[:, :, c * RS:(c + 1) * RS] = blk.reshape(T, L, RS)
    out_spks = np.ascontiguousarray(results[0]["ospk"].T)  # [T, OUT]
    return out_spks, mem_his


_RUN_CACHE = {}


def kernel(inp, fc0, fc1, fc2, out_w, target=None, bf=None, **_unused):
    from concourse import bass_utils

    if "nc" not in _RUN_CACHE:
        _RUN_CACHE["nc"] = _build_program()
    nc = _RUN_CACHE["nc"]
    in_maps = _host_inputs(inp, fc0, fc1, fc2, out_w)
    res = bass_utils.run_bass_kernel_spmd(nc, in_maps, list(range(NCORES)))
    return _assemble(res.results)
